# revision 27
# baseline (speedup 1.0000x reference)
"""DeepseekMoE block-quantized MoE kernel for 8 Trainium2 NeuronCores.

Strategy (expert-parallel with host-side dispatch):
  - The routing table (selected_experts) is known on the host before launch,
    so the all-to-all "dispatch" is done on the host: for each expert e we
    gather the unique tokens routed to it (dedup across the top-k slots),
    transpose to [H, n_e], and pad to the slot capacity.
  - Work is packed into 5 SPMD slots (widths ~[836,472,456,448,428], total
    2640 cols/core vs the 2629 floor); a small feasibility solver maps the
    16 experts onto the 40 (core, slot) bins, splitting an expert's token
    list across bins where needed.  Fallbacks: the 4-slot pair-cover solver,
    then a 2-slot scheme.
  - Each slot runs a dense 3-matmul MLP (gate/up -> silu*up -> down) in
    x^T / act^T layout so no on-device transposes are needed.
  - Block-dequantization (w * repeat(s, 128)) is folded into the host-side
    weight preparation, which also emits slab-contiguous weight layouts so
    every weight DMA is a pure linear copy (4KB per partition line).
  - All matmul operands are bf16: same 1 col/cycle PE rate as fp32r, but
    half the HBM traffic.  fp8 was evaluated and rejected twice: e4m3
    activation quantization alone puts rel_l2 at ~4.6e-2 (gate 2e-2), and
    DoubleRow doubles the contraction dim, so a 3-term error-compensated
    fp8 scheme costs 1.5x the bf16 cycle count - strictly worse.
  - DMA queue plan (trace-driven): current-job w0/w1 slabs and w2 slabs ride
    the sync HWDGE ring; x tiles and y writebacks ride the scalar HWDGE
    ring; next-job w0/w1 prefetch rides the gpsimd SWDGE ring.  Next-job
    prefetch is spread across phase A i-tiles (the phase-B window is DMA-
    oversubscribed if prefetch lands there - measured 2x ~3.4us stalls).
  - ~96 dummy 64-col matmuls on a memset tile run during the ~10us runtime
    preamble so the PE's HAM clock gate (cold 1.2GHz -> warm 2.4GHz after
    ~3.4us of sustained activity) is already warm when real matmuls start.
  - The first job's i=0 weight slabs are split into pieces so the first
    real matmul only waits on a 256KB + x-tile transfer.
  - The host scatters the per-expert outputs back to [T, K, H].
"""

import math
import time as _time

import numpy as np

T = 4096
TOPK = 6
E = 16
H = 2048
I = 1408
BS = 128           # quant block size
HT = H // 128      # 16 h-tiles
IT = I // 128      # 11 i-tiles
NCORES = 8
# Single-pass width bound: at most 4 PSUM-bank chunks (the interleaved
# accumulation rings are 4 deep), and SBUF capacity bounds total width.
MAX_W = 2040
N_DUMMY = 100      # HAM warm-up matmuls during the runtime preamble

_BUILT = {}
LAST_RESULTS = None  # stashed BassKernelResults for external harnesses


def _chunk_plan(width, small_first=False):
    """Split `width` columns into PSUM-bank-sized chunks (<=512), each >=256
    when width allows (small free dims pay LDWEIGHTS/dispatch overhead)."""
    if width <= 512:
        return [(0, width)]
    if small_first and width > 768:
        return [(0, 256)] + [(256 + o, w) for o, w in _chunk_plan(width - 256)]
    n = -(-width // 512)
    # 8-aligned chunk widths
    base = (width // n) // 8 * 8
    rem8 = (width - n * base) // 8
    out, off = [], 0
    for j in range(n):
        w = base + (8 if j < rem8 else 0)
        if j == n - 1:
            w = width - off
        out.append((off, w))
        off += w
    return out


def _feasible5(widths, counts, tries, rng):
    """Assign each expert a multiset of width-classes (8 bins per class,
    one bin = one (core, slot)) with sum >= count, via bounded DFS with
    backtracking.  Returns {expert: (w1, w2[, w3])} or None."""
    import itertools
    from collections import Counter

    classes = Counter(widths)
    order = sorted(range(len(counts)), key=lambda e: -counts[e])
    need_suffix = [0] * (len(order) + 1)
    for i in range(len(order) - 1, -1, -1):
        need_suffix[i] = need_suffix[i + 1] + int(counts[order[i]])
    budget = [max(2000, tries * 100)]
    memo_fail = set()

    def dfs(idx, avail):
        if idx == len(order):
            return {}
        if budget[0] <= 0:
            return None
        budget[0] -= 1
        key = (idx, tuple(sorted(avail.items())))
        if key in memo_fail:
            return None
        cap = sum(w * c for w, c in avail.items())
        if cap < need_suffix[idx]:
            memo_fail.add(key)
            return None
        ne = int(counts[order[idx]])
        ws = [w for w in avail if avail[w] > 0]
        combos = []
        for r in (1, 2, 3):
            for c in itertools.combinations_with_replacement(ws, r):
                cc = Counter(c)
                if all(avail[w] >= k for w, k in cc.items()):
                    s = sum(c)
                    if s >= ne:
                        combos.append((s - ne, len(c), c))
        combos.sort()
        for _, _, c in combos[:6]:
            av2 = dict(avail)
            for w in c:
                av2[w] -= 1
            sub = dfs(idx + 1, av2)
            if sub is not None:
                sub[order[idx]] = c
                return sub
        memo_fail.add(key)
        return None

    return dfs(0, {w: 8 * m for w, m in classes.items()})


def _solve5(counts):
    """Find a slot-width multiset (4-6 slots) + expert assignment with
    small total width.  Returns (widths_in_job_order, plan) or None where
    plan[c][s] = (expert, tok_off, ncols)."""
    import random

    rng = random.Random(0)
    total = int(np.sum(counts))
    lb = -(-total // NCORES)
    best = None

    def try_widths(ws):
        nonlocal best
        ws = tuple(sorted(ws, reverse=True))
        if best is not None and sum(ws) >= best[0]:
            return
        if min(ws) < 400 or max(ws) > MAX_W:
            return
        a = _feasible5(list(ws), counts, 24, rng)
        if a is not None:
            best = (sum(ws), ws, a)

    # known-good multisets for the reference routing table (jax PRNG differs
    # between platforms, so both observed count vectors are covered), then
    # random search as a safety net for unexpected routings
    try_widths((856, 472, 460, 432, 412))
    try_widths((836, 472, 456, 448, 428))
    t0 = _time.time()
    while _time.time() - t0 < 2.0:
        S = rng.choice((5, 5, 5, 6))
        tgt = (best[0] if best else lb + 44) - 4
        rem = tgt // 4 - S * 100  # compose tgt/4 into S parts each >= 100
        if rem < 0:
            break
        bars = sorted(rng.sample(range(rem + S - 1), S - 1)) if S > 1 else []
        xs2 = [-1] + bars + [rem + S - 1]
        ws = [(xs2[i + 1] - xs2[i] - 1 + 100) * 4 for i in range(S)]
        if sum(ws) == tgt:
            try_widths(ws)
    if best is None:
        return None
    _, ws, assign = best
    # piece allocation: expert e gets pieces of sizes from its width multiset
    from collections import defaultdict

    class_pieces = defaultdict(list)  # width -> [(expert, off, n)]
    for e, combo in assign.items():
        ne = int(counts[e])
        off = 0
        for w in sorted(combo, reverse=True):
            n = min(ne - off, w)
            class_pieces[w].append((e, off, n))
            off += n
    # job order: 2nd-smallest first, smallest last, rest descending between
    ws_sorted = sorted(ws)
    order = [ws_sorted[1]] + sorted(ws_sorted[2:], reverse=False) + [ws_sorted[0]]
    # map job order to distinct class copies
    used = defaultdict(int)
    widths_job = []
    class_of_job = []
    for w in order:
        widths_job.append(w)
        class_of_job.append((w, used[w]))
        used[w] += 1
    # distribute class pieces to cores: for width class w, its pieces fill
    # jobs of that class in round-robin over cores
    plan = [[None] * len(widths_job) for _ in range(NCORES)]
    jobs_of_class = defaultdict(list)
    for j, (w, k) in enumerate(class_of_job):
        jobs_of_class[w].append(j)
    for w, pieces in class_pieces.items():
        slots = [(c, j) for j in jobs_of_class[w] for c in range(NCORES)]
        assert len(pieces) <= len(slots)
        for (c, j), (e, off, n) in zip(slots, pieces):
            plan[c][j] = (e, off, n)
    for c in range(NCORES):
        for j in range(len(widths_job)):
            if plan[c][j] is None:
                plan[c][j] = (0, 0, 0)
    return widths_job, plan


def _solve4(counts):
    """4-job pair-cover solver (fallback).  See baseline docstring."""
    import itertools

    order = np.argsort(-counts, kind="stable")
    pairs = [
        (int(counts[order[i]]), int(counts[order[2 * NCORES - 1 - i]]))
        for i in range(NCORES)
    ]
    best = None
    cands = []
    for assign in itertools.product(range(3), repeat=len(pairs)):
        req = [[0, 0], [0, 0], [0, 0]]
        for (a, b), p in zip(pairs, assign):
            req[p][0] = max(req[p][0], a)
            req[p][1] = max(req[p][1], b)
        lb = max(
            max(ra + rb for ra, rb in req),
            -(-sum(ra + rb for ra, rb in req) // 3),
        )
        cands.append((lb, assign, req))
    cands.sort(key=lambda t: t[0])
    for lb, assign, req in cands[:200]:
        (m1a, m1b), (m2a, m2b), (m3a, m3b) = req
        for A in range(256, 1500, 4):
            B = max(m1a - A, 256)
            C = max(m2a - A, m3b - B, 256)
            D = max(m3a - A, m2b - B, 256)
            if C + D < m1b:
                D += m1b - (C + D)
            w = [-(-v // 4) * 4 for v in (A, B, C, D)]
            S = sum(w)
            if best is None or S < best[0]:
                best = (S, w, assign)
        if best is not None and best[0] <= lb:
            break
    if best is None:
        return None
    S, w, assign = best
    parts = {0: ((0, 1), (2, 3)), 1: ((0, 2), (1, 3)), 2: ((0, 3), (1, 2))}
    cover = []
    for (a, b), p in zip(pairs, assign):
        big, small = parts[p]
        if w[big[0]] + w[big[1]] < a or w[small[0]] + w[small[1]] < b:
            return None
        cover.append((big, small))
    experts_of_core = [
        (int(order[i]), int(counts[order[i]]),
         int(order[2 * NCORES - 1 - i]), int(counts[order[2 * NCORES - 1 - i]]))
        for i in range(NCORES)
    ]
    return w, cover, experts_of_core


def _build(jobs, CT):
    """Build the SPMD Bass program.  `jobs` is a tuple of
    (slot, col_offset, width): each job runs one expert slot's MLP over a
    window of `width` token columns; CT is the column capacity of xt/yt."""
    import concourse.bacc as bacc
    import concourse.mybir as mybir
    from concourse.bass import ts
    from concourse.tile import TileContext

    f32 = mybir.dt.float32
    bf16 = mybir.dt.bfloat16
    AF = mybir.ActivationFunctionType
    import os as _os

    act_fn = (
        AF.Sigmoid if _os.environ.get("KERNEL_SIM_SIGMOID") else AF.Silu
    )  # CoreSim lacks Silu; HW path always uses Silu

    NS = max(j[0] for j in jobs) + 1
    nc = bacc.Bacc()
    # xt is partition-major so 4 h-tiles load as ONE dma_start (HWDGE trigger
    # instructions cost ~0.7us of issuing-engine time each; 16 per job
    # serialized on the scalar engine starved the fill and the activations)
    xt = nc.declare_dram_parameter("xt", [NS, 128, HT, CT], bf16, isOutput=False)
    # slab-contiguous weights: w0t/w1t slab i = [128, H]; w2t slab h = [128, I]
    w0t = nc.declare_dram_parameter("w0t", [NS, IT, 128, H], bf16, isOutput=False)
    w1t = nc.declare_dram_parameter("w1t", [NS, IT, 128, H], bf16, isOutput=False)
    w2t = nc.declare_dram_parameter("w2t", [NS, HT, 128, I], bf16, isOutput=False)
    yt = nc.declare_dram_parameter("yt", [NS, HT, 128, CT], bf16, isOutput=True)

    with TileContext(nc) as tc:
        with (
            tc.tile_pool(name="xp", bufs=1) as xp,
            tc.tile_pool(name="ap", bufs=1) as apool,
            tc.tile_pool(name="wp", bufs=2) as wp,
            tc.tile_pool(name="yp", bufs=6) as yp,
            tc.tile_pool(name="dp", bufs=1) as dp,
            tc.tile_pool(name="ps", bufs=3, space="PSUM") as ps,
        ):
            # ---- HAM warm-up: dummy matmuls during the runtime preamble ----
            warm = dp.tile([128, 128], bf16, tag="warm")
            nc.gpsimd.memset(warm[:], 0.0)
            wps = ps.tile([128, 512], f32, tag="g", bufs=4, name="warmps")
            for _ in range(N_DUMMY):
                nc.tensor.matmul(
                    wps[:, :64], warm[:, :128], warm[:, :64],
                    start=True, stop=True,
                )

            def load_w01_slab(which, src, s, i, queue=None, pieces=1):
                slab = wp.tile([128, H], bf16, tag=which, name=None, bufs=4)
                q = queue if queue is not None else nc.sync
                if pieces == 1:
                    q.dma_start(out=slab, in_=src[s, i])
                else:
                    step = H // pieces
                    for p in range(pieces):
                        q.dma_start(
                            out=slab[:, p * step : (p + 1) * step],
                            in_=src[s, i, :, p * step : (p + 1) * step],
                        )
                return slab

            G = 4  # h-tiles per x group DMA

            def emit_xg(xs, s, co, W, g, queue):
                # one trigger loads h-tiles 4g..4g+3: [128, 4, W]
                queue.dma_start(
                    out=xs[g][:, :, :W],
                    in_=xt[s, :, G * g : G * (g + 1), co : co + W],
                )

            def x_rhs(xs, h, c0, cw):
                return xs[h // G][:, h % G, c0 : c0 + cw]

            def make_xtiles(jn):
                return [
                    xp.tile([128, G, CT], bf16, tag=f"xg{g}", name=f"xg{g}_{jn}",
                            bufs=2)
                    for g in range(HT // G)
                ]

            # ---- first job fill: x groups on the scalar HWDGE ring; the
            # first 3 i-tiles' weight slabs are loaded in half-slab pieces,
            # first halves (h-tiles 0-7) of all six slabs before any second
            # half, so the joint i={0,1,2} sweep can start on ~2 transfers ----
            s0, co0, W0 = jobs[0]
            xs0 = make_xtiles(0)
            fill_slabs = {
                i: [
                    wp.tile([128, H], bf16, tag=which, name=f"{which}f{i}",
                            bufs=4)
                    for which in ("w0", "w1")
                ]
                for i in range(3)
            }
            # spread the fill across all three DMA paths so packet-level
            # round-robin approximates the PE's consumption order: per-h x
            # pieces on gpsimd (subtile deps let MM h wait on piece h only),
            # w0 half-slabs on sync, w1 half-slabs on scalar.
            for h in range(HT):
                nc.gpsimd.dma_start(
                    out=xs0[h // G][:, h % G, :W0],
                    in_=xt[s0, :, h, co0 : co0 + W0],
                )
            fill_pieces = ((0, 512), (512, 1024), (1024, 2048))
            for p0, p1 in fill_pieces:
                for i in range(3):
                    piece = slice(p0, p1)
                    nc.sync.dma_start(
                        out=fill_slabs[i][0][:, piece], in_=w0t[s0, i, :, piece]
                    )
                for i in range(3):
                    piece = slice(p0, p1)
                    nc.scalar.dma_start(
                        out=fill_slabs[i][1][:, piece], in_=w1t[s0, i, :, piece]
                    )
            slab_q0 = {i: fill_slabs[i] for i in range(3)}
            pre = (xs0, slab_q0)

            for jn, (s, co, W) in enumerate(jobs):
                    chunks = _chunk_plan(W)
                    xs, slab_q = pre
                    acts = [
                        apool.tile([128, CT], bf16, tag=f"a{i}", name=f"a{i}_{jn}")
                        for i in range(IT)
                    ]
                    have_next = jn + 1 < len(jobs)
                    if have_next:
                        sn, con, Wn = jobs[jn + 1]
                        xs_next = make_xtiles(jn + 1)
                        slab_q_next = {}

                    # Phase A: gate/up projections + silu*up, per i-tile.
                    # h-outer with chunk-interleaved PSUM accumulation: each
                    # xs[h] is consumed exactly once (streams at DMA delivery
                    # pace during the fill) and consecutive matmuls share the
                    # stationary weight tile.  Next-job prefetch (x and w0/w1
                    # i-tiles 0-2 on gpsimd) is spread across i-tiles so the
                    # phase-B DMA window stays under the HBM-per-core cap.
                    #
                    # Job 0 starts with a JOINT sweep over i-tiles 0-2 (6 PSUM
                    # banks, h-outer): the kernel fill is HBM-bound (x + six
                    # slabs before i=0 can finish), so tripling the PE work
                    # per delivered x byte keeps the PE busy during the fill
                    # instead of idling >3.4us and re-throttling the clock.
                    i_start = 0
                    if jn == 0 and len(chunks) == 1 and IT > 5:
                        group = [0, 1, 2]
                        slabs = [slab_q.pop(i) for i in group]
                        gbs = [
                            ps.tile([128, 512], f32, tag="g", bufs=4,
                                    name=f"g{jn}_{i}_0")
                            for i in group
                        ]
                        ubs = [
                            ps.tile([128, 512], f32, tag="u", bufs=4,
                                    name=f"u{jn}_{i}_0")
                            for i in group
                        ]
                        W_ = chunks[0][1]
                        for h in range(HT):
                            for gi, i in enumerate(group):
                                nc.tensor.matmul(
                                    gbs[gi][:, :W_],
                                    slabs[gi][0][:, ts(h, 128)],
                                    x_rhs(xs, h, 0, W_),
                                    start=(h == 0),
                                    stop=(h == HT - 1),
                                )
                                nc.tensor.matmul(
                                    ubs[gi][:, :W_],
                                    slabs[gi][1][:, ts(h, 128)],
                                    x_rhs(xs, h, 0, W_),
                                    start=(h == 0),
                                    stop=(h == HT - 1),
                                )
                            if h in (6, 10, 14) and 3 + (h - 6) // 4 < IT:
                                i_ld = 3 + (h - 6) // 4
                                slab_q[i_ld] = [
                                    load_w01_slab("w0", w0t, s, i_ld),
                                    load_w01_slab("w1", w1t, s, i_ld),
                                ]
                        for gi, i in enumerate(group):
                            a_sl = acts[i][:, :W_]
                            nc.scalar.activation(a_sl, gbs[gi][:, :W_], act_fn)
                            nc.vector.tensor_mul(a_sl, a_sl, ubs[gi][:, :W_])
                        i_start = 3
                    n_pre = max(slab_q) + 1 - min(slab_q)
                    for i in range(i_start, IT):
                        w0s, w1s = slab_q.pop(i)
                        i_next = i + n_pre
                        if i_next < IT:
                            slab_q[i_next] = [
                                load_w01_slab("w0", w0t, s, i_next),
                                load_w01_slab("w1", w1t, s, i_next),
                            ]
                        if have_next:
                            if i in (3, 5, 7, 9):
                                emit_xg(xs_next, sn, con, Wn, (i - 3) // 2, nc.gpsimd)
                            if i in (4, 6, 8):
                                i2 = (i - 4) // 2
                                slab_q_next[i2] = [
                                    load_w01_slab("w0", w0t, sn, i2, queue=nc.gpsimd),
                                    load_w01_slab("w1", w1t, sn, i2, queue=nc.gpsimd),
                                ]
                        # current-job w2 prefetch late in phase A (sync ring,
                        # behind the last w0/w1 slabs); 5 deep so the in-loop
                        # triggers' WAR waits never gate the PE
                        if i == IT - 3:
                            w2s_pre = [
                                wp.tile([128, I], bf16, tag="w2", bufs=8,
                                        name=f"w2_{jn}_{h}")
                                for h in range(7)
                            ]
                            for h in range(7):
                                nc.sync.dma_start(out=w2s_pre[h], in_=w2t[s, h])
                        gb = [ps.tile([128, 512], f32, tag="g", bufs=4, name=f"g{jn}_{i}_{ci}") for ci in range(len(chunks))]
                        ub = [ps.tile([128, 512], f32, tag="u", bufs=4, name=f"u{jn}_{i}_{ci}") for ci in range(len(chunks))]
                        for h in range(HT):
                            for ci, (c0, cw) in enumerate(chunks):
                                nc.tensor.matmul(
                                    gb[ci][:, :cw],
                                    w0s[:, ts(h, 128)],
                                    x_rhs(xs, h, c0, cw),
                                    start=(h == 0),
                                    stop=(h == HT - 1),
                                )
                            for ci, (c0, cw) in enumerate(chunks):
                                nc.tensor.matmul(
                                    ub[ci][:, :cw],
                                    w1s[:, ts(h, 128)],
                                    x_rhs(xs, h, c0, cw),
                                    start=(h == 0),
                                    stop=(h == HT - 1),
                                )
                        for ci, (c0, cw) in enumerate(chunks):
                            a_sl = acts[i][:, c0 : c0 + cw]
                            nc.scalar.activation(a_sl, gb[ci][:, :cw], act_fn)
                            nc.vector.tensor_mul(a_sl, a_sl, ub[ci][:, :cw])

                    if have_next:
                        pre = (xs_next, slab_q_next)
                        # remaining slab prefetch depth handled by next job's
                        # phase A loop (n_pre = 3)

                    # Phase B: down projection, per h-tile, i-outer with the
                    # same chunk interleaving (PSUM banks shared with the "u"
                    # ring).  w2 slabs ride the sync ring (prefetched 3 deep),
                    # y writebacks the scalar ring.
                    for h in range(HT):
                        if h < 7:
                            w2s = w2s_pre[h]
                        else:
                            w2s = wp.tile([128, I], bf16, tag="w2", bufs=8,
                                          name=f"w2_{jn}_{h}")
                            nc.sync.dma_start(out=w2s, in_=w2t[s, h])
                        yc = yp.tile([128, CT], bf16, tag="y", bufs=5)
                        ob = [ps.tile([128, 512], f32, tag="u", bufs=4, name=f"o{jn}_{h}_{ci}") for ci in range(len(chunks))]
                        for i in range(IT):
                            for ci, (c0, cw) in enumerate(chunks):
                                nc.tensor.matmul(
                                    ob[ci][:, :cw],
                                    w2s[:, ts(i, 128)],
                                    acts[i][:, c0 : c0 + cw],
                                    start=(i == 0),
                                    stop=(i == IT - 1),
                                )
                        if jn == len(jobs) - 1 and h == HT - 1 and len(chunks) == 1:
                            # final writeback: split so the first half's cast
                            # and DMA overlap the second half's cast
                            hw_ = W // 2
                            for c0_, cw_ in ((0, hw_), (hw_, W - hw_)):
                                nc.vector.tensor_copy(
                                    yc[:, c0_ : c0_ + cw_], ob[0][:, c0_ : c0_ + cw_]
                                )
                                nc.scalar.dma_start(
                                    out=yt[s, h, :, co + c0_ : co + c0_ + cw_],
                                    in_=yc[:, c0_ : c0_ + cw_],
                                )
                        else:
                            for ci, (c0, cw) in enumerate(chunks):
                                nc.vector.tensor_copy(yc[:, c0 : c0 + cw], ob[ci][:, :cw])
                            nc.scalar.dma_start(
                                out=yt[s, h, :, co : co + W], in_=yc[:, :W]
                            )
    nc.finalize()
    return nc


def _get_built(jobs, CT):
    key = (tuple(jobs), CT)
    if key not in _BUILT:
        _BUILT[key] = _build(tuple(jobs), CT)
    return _BUILT[key]


def _dequant(w, s):
    """w: [E, O, Iin], s: [E, O, Iin//128] -> dequantized [E, O, Iin]."""
    e, o, iin = w.shape
    nb = -(-iin // BS)
    if nb * BS != iin:
        s_full = np.repeat(s, BS, axis=-1)[..., :iin]
        return w * s_full
    return (w.reshape(e, o, nb, BS) * s[..., None]).reshape(e, o, iin)


def _slabify(wd, bf16):
    """wd: [E, O, C] dequantized weights -> [E, O//128, 128, C] bf16 where
    slab o = [128 c-sub partitions, O-tile columns grouped by c-tile]:
    out[e, o, p, ct*128+j] = wd[e, o*128+j, ct*128+p]."""
    e, o, c = wd.shape
    ot, ct = o // 128, c // 128
    v = wd.astype(bf16).reshape(e, ot, 128, ct, 128)
    return v.transpose(0, 1, 4, 3, 2).reshape(e, ot, 128, c)


def kernel(**inputs):
    global LAST_RESULTS
    import ml_dtypes

    bf16 = ml_dtypes.bfloat16

    x = np.ascontiguousarray(np.asarray(inputs["x"], dtype=np.float32))
    sel = np.asarray(inputs["selected_experts"])
    w0 = np.asarray(inputs["w0"], dtype=np.float32)
    s0 = np.asarray(inputs["s0"], dtype=np.float32)
    w1 = np.asarray(inputs["w1"], dtype=np.float32)
    s1 = np.asarray(inputs["s1"], dtype=np.float32)
    w2 = np.asarray(inputs["w2"], dtype=np.float32)
    s2 = np.asarray(inputs["s2"], dtype=np.float32)

    t, k = sel.shape
    assert (t, k) == (T, TOPK) and x.shape == (T, H)

    # ---- host-side dispatch: unique tokens per expert ----
    pos = np.full((E, T), -1, dtype=np.int32)
    cols = []
    for e in range(E):
        toks = np.nonzero((sel == e).any(axis=1))[0]
        cols.append(toks)
        pos[e, toks] = np.arange(len(toks), dtype=np.int32)
    counts = np.array([len(c) for c in cols])

    def align4(v):
        return max(256, -(-v // 4) * 4)

    order = np.argsort(-counts, kind="stable")
    two_slot_total = align4(int(counts[order[0]])) + align4(int(counts[order[NCORES]]))

    plan = None  # plan[c] = list over slots of (expert, tok_off, ncols)
    sol5 = _solve5(counts)
    sol4 = _solve4(counts)
    sol4_total = sum(sol4[0]) if sol4 is not None else 1 << 30
    if (
        sol5 is not None
        and sum(sol5[0]) < min(sol4_total, two_slot_total)
        and max(sol5[0]) <= MAX_W
    ):
        widths, plan = sol5
        jobs = tuple((j, 0, int(w)) for j, w in enumerate(widths))
        CT = max(widths)
        NS = len(widths)
    elif sol4 is not None and sol4_total < two_slot_total and max(sol4[0]) <= MAX_W:
        w4, cover, experts_of_core = sol4
        # emit jobs in ascending width order (smallest first: cheaper fill)
        jobs = tuple((int(j), 0, int(w4[j])) for j in np.argsort(w4, kind="stable"))
        CT = max(w4)
        NS = 4
        plan = []
        for c in range(NCORES):
            ea, na, eb, nb = experts_of_core[c]
            big, small = cover[c]
            slots = [None] * NS
            for e, n, jl in ((ea, na, big), (eb, nb, small)):
                n0 = min(n, w4[jl[0]])
                slots[jl[0]] = (e, 0, n0)
                slots[jl[1]] = (e, n0, n - n0)
            plan.append(slots)
    else:
        # 2-slot scheme: slot 0 the 8 largest experts, slot 1 the 8 smallest
        expert_of = [list(order[:NCORES]), list(order[NCORES:])]
        slot_w = [align4(int(counts[expert_of[s]].max())) for s in range(2)]
        NS = 2
        if max(slot_w) <= MAX_W:
            jobs = tuple((s, 0, slot_w[s]) for s in range(2))
            CT = max(slot_w)
        else:
            cmax = int(counts.max())
            passes = max(1, math.ceil(cmax / MAX_W))
            W = align4(math.ceil(cmax / passes))
            CT = W * passes
            jobs = tuple((s, cp * W, W) for s in range(2) for cp in range(passes))
        plan = []
        for c in range(NCORES):
            plan.append(
                [(int(expert_of[s][c]), 0, int(counts[expert_of[s][c]]))
                 for s in range(2)]
            )

    NS = max(j[0] for j in jobs) + 1

    # ---- dequantize + slabify weights (host) ----
    # w0/w1: [E, I, H] -> slabs [E, IT, 128, H]; w2: [E, H, I] -> [E, HT, 128, I]
    w0s_all = _slabify(_dequant(w0, s0), bf16)
    w1s_all = _slabify(_dequant(w1, s1), bf16)
    w2s_all = _slabify(_dequant(w2, s2), bf16)

    x_bf = x.astype(bf16)
    in_maps = []
    for c in range(NCORES):
        # partition-major: xt_c[s, p, h, col] = x^T[h*128+p, col]
        xt_c = np.zeros((NS, 128, HT, CT), dtype=bf16)
        exps = []
        for s, (e, off, n) in enumerate(plan[c]):
            exps.append(e)
            if n:
                xs_piece = x_bf[cols[e][off : off + n]].T  # [H, n]
                xt_c[s, :, :, :n] = xs_piece.reshape(HT, 128, n).transpose(1, 0, 2)
        in_maps.append(
            {
                "xt": xt_c,
                "w0t": np.ascontiguousarray(w0s_all[exps]),
                "w1t": np.ascontiguousarray(w1s_all[exps]),
                "w2t": np.ascontiguousarray(w2s_all[exps]),
            }
        )

    nc = _get_built(jobs, CT)
    from concourse.bass_utils import run_bass_kernel_spmd

    res = run_bass_kernel_spmd(nc, in_maps, list(range(NCORES)))
    LAST_RESULTS = res

    # Y[e] = [H, n_e] for expert e (token order = cols[e])
    Y = np.zeros((E, H, int(counts.max())), dtype=np.float32)
    for c in range(NCORES):
        yt_c = np.asarray(res.results[c]["yt"]).astype(np.float32).reshape(NS, H, CT)
        for s, (e, off, n) in enumerate(plan[c]):
            if n:
                Y[e][:, off : off + n] = yt_c[s][:, :n]

    # ---- scatter back to [T, K, H] ----
    e_flat = sel.reshape(-1).astype(np.int64)
    t_flat = np.repeat(np.arange(T, dtype=np.int64), TOPK)
    p_flat = pos[e_flat, t_flat]
    out = Y[e_flat, :, p_flat]  # [T*K, H]
    return np.ascontiguousarray(out.reshape(T, TOPK, H), dtype=np.float32)
